# revision 6
# baseline (speedup 1.0000x reference)
"""Trainium2 Bass kernel for nn_PhysicsGuidedGCN.

Reference computation (shapes: x [B=4, N=256, T=128, F=64], A_list [K=8, N, N]):
    A    = row_normalize(A_list)
    h    = x @ in_w + in_b                      # [B,N,T,H]
    alpha= tiny gating MLP over (lag_embed, mean(x))   # [B,K] scalars
    out  = sum_k alpha[:,k] * A[k] @ shift_k(h)  (shift along T, zero fill)
    out  = out @ out_w + out_b
    res  = out + h ; LayerNorm(H) ; gelu        # returns [B,N,T,H]

Device strategy (8 cores, data-parallel over B x T-halves; core = b*2 + half):
  Fold out_w left: out = sum_k alpha_k A_k shift_k(x @ W2) + corr, W2 = in_w@out_w.
  Per core (local T range TL=64 plus HALO=7 backward halo):
    phase 1: p = xT.T @ W2 per timestep  -> p_sb [node, (t,h)] fp16 in SBUF
    phase 2: per output chunk [128 nodes x 2 t x 256 h] accumulate in PSUM:
             16 lag matmuls (lhsT = alpha-scaled A^T slices, rhs = shifted p cols)
             + 2 residual matmuls (x @ in_w)
             then + cvec (all bias terms incl. row-sum identity of A), LayerNorm,
             gelu, DMA out.
  The tiny gating MLP (32 scalars) and A normalization/scaling run on host;
  they are O(KN^2 + B*F) and replicated per the sharding hint.
"""

import numpy as np
from contextlib import ExitStack

import concourse.bass as bass
import concourse.mybir as mybir
import concourse.tile as tile
from concourse import bacc
from concourse.bass_utils import run_bass_kernel_spmd

B, N, T, F, K, H = 4, 256, 128, 64, 8, 256
TL = 64           # timesteps per core
HALO = K - 1      # backward halo
TP = TL + HALO    # 71 timesteps of p per core
NTC = TP * N      # p_sb columns (t-major, 256 per t)
NCORES = 8
FP16 = mybir.dt.float16
FP32 = mybir.dt.float32
AF = mybir.ActivationFunctionType
OP = mybir.AluOpType


def build_nc(act_func=AF.Gelu):
    nc = bacc.Bacc("TRN2", target_bir_lowering=False)
    xT = nc.dram_tensor("xT", [F, NTC], FP16, kind="ExternalInput")
    ahatT = nc.dram_tensor("ahatT", [N, K * N], FP16, kind="ExternalInput")
    w2 = nc.dram_tensor("w2", [F, H], FP16, kind="ExternalInput")
    inw = nc.dram_tensor("inw", [F, H], FP16, kind="ExternalInput")
    cvec = nc.dram_tensor("cvec", [128, 16 * H], FP32, kind="ExternalInput")
    gvec = nc.dram_tensor("gvec", [128, 2 * H], FP32, kind="ExternalInput")
    bvec = nc.dram_tensor("bvec", [128, 2 * H], FP32, kind="ExternalInput")
    out = nc.dram_tensor("out", [N, TL, H], FP32, kind="ExternalOutput")

    with tile.TileContext(nc) as tc, ExitStack() as ctx:
        singles = ctx.enter_context(tc.tile_pool(name="singles", bufs=1))
        psum = ctx.enter_context(tc.tile_pool(name="psum", bufs=8, space="PSUM"))
        epool = ctx.enter_context(tc.tile_pool(name="epool", bufs=3))
        spool = ctx.enter_context(tc.tile_pool(name="spool", bufs=4))
        opool = ctx.enter_context(tc.tile_pool(name="opool", bufs=3))

        xT_sb = singles.tile([F, NTC], FP16)
        nc.sync.dma_start(out=xT_sb, in_=xT[:, :])
        a_sb = []
        for jc in range(2):
            a = singles.tile([128, K * N], FP16, name=f"a_sb{jc}", tag=f"a_sb{jc}")
            nc.sync.dma_start(out=a, in_=ahatT[jc * 128:(jc + 1) * 128, :])
            a_sb.append(a)
        w2_sb = singles.tile([F, H], FP16)
        nc.sync.dma_start(out=w2_sb, in_=w2[:, :])
        inw_sb = singles.tile([F, H], FP16)
        nc.sync.dma_start(out=inw_sb, in_=inw[:, :])
        c_sb = singles.tile([128, 16 * H], FP32)
        nc.sync.dma_start(out=c_sb, in_=cvec[:, :])
        g_sb = singles.tile([128, 2 * H], FP32)
        nc.sync.dma_start(out=g_sb, in_=gvec[:, :])
        b_sb = singles.tile([128, 2 * H], FP32)
        nc.sync.dma_start(out=b_sb, in_=bvec[:, :])
        eps_sb = singles.tile([128, 1], FP32)
        nc.vector.memset(eps_sb, 1e-5)
        p_sb = [
            singles.tile([128, NTC], FP16, name=f"p_sb{jc}", tag=f"p_sb{jc}")
            for jc in range(2)
        ]

        # phase 1: p[n, (t,h)] = x[n,t,:] @ W2, one matmul per (t, node-chunk)
        for tp in range(TP):
            for nc2 in range(2):
                ps = psum.tile([128, 512], FP32, name="ps", tag="ps")
                nc.tensor.matmul(
                    ps[:, :H],
                    lhsT=xT_sb[:, tp * N + nc2 * 128: tp * N + nc2 * 128 + 128],
                    rhs=w2_sb[:, :],
                    start=True,
                    stop=True,
                )
                nc.scalar.activation(
                    out=p_sb[nc2][:, tp * H:(tp + 1) * H],
                    in_=ps[:, :H],
                    func=AF.Copy,
                )

        # phase 2: output chunks [128 nodes, 2 t, 256 h]
        for tch in range(TL // 2):
            for ic in range(2):
                ps3 = psum.tile([128, 512], FP32, name="ps3", tag="ps")
                # accumulation group: k=0 lags (start) -> residual x@in_w
                # sub-range matmuls -> k=1..7 lags (stop on last, full width)
                for jc in range(2):
                    col = (2 * tch + HALO) * H
                    nc.tensor.matmul(
                        ps3,
                        lhsT=a_sb[jc][:, ic * 128: ic * 128 + 128],
                        rhs=p_sb[jc][:, col: col + 512],
                        start=(jc == 0),
                        stop=False,
                    )
                for trel in range(2):
                    tloc = 2 * tch + trel
                    nc.tensor.matmul(
                        ps3[:, trel * H:(trel + 1) * H],
                        lhsT=xT_sb[:, (tloc + HALO) * N + ic * 128:
                                   (tloc + HALO) * N + ic * 128 + 128],
                        rhs=inw_sb[:, :],
                        start=False,
                        stop=False,
                        skip_group_check=True,
                    )
                for k in range(1, K):
                    for jc in range(2):
                        col = (2 * tch + HALO - k) * H
                        nc.tensor.matmul(
                            ps3,
                            lhsT=a_sb[jc][:, k * N + ic * 128: k * N + ic * 128 + 128],
                            rhs=p_sb[jc][:, col: col + 512],
                            start=False,
                            stop=(k == K - 1 and jc == 1),
                        )

                ccol = 2 * tch * H if tch < 8 else 14 * H
                res = epool.tile([128, 512], FP32, name="res", tag="res")
                nc.vector.tensor_tensor(
                    out=res, in0=ps3, in1=c_sb[:, ccol: ccol + 512], op=OP.add
                )
                stats = spool.tile([128, 2, 6], FP32, name="stats", tag="stats")
                mv = spool.tile([128, 2, 2], FP32, name="mv", tag="mv")
                for s2 in range(2):
                    nc.vector.bn_stats(out=stats[:, s2], in_=res[:, s2 * H:(s2 + 1) * H])
                    nc.vector.bn_aggr(out=mv[:, s2], in_=stats[:, s2])
                std = spool.tile([128, 2], FP32, name="std", tag="std")
                nc.scalar.activation(
                    out=std, in_=mv[:, :, 1], func=AF.Sqrt, bias=eps_sb, scale=1.0
                )
                rstd = spool.tile([128, 2], FP32, name="rstd", tag="rstd")
                nc.vector.reciprocal(out=rstd, in_=std)
                norm = epool.tile([128, 512], FP32, name="norm", tag="norm")
                for s2 in range(2):
                    nc.vector.tensor_scalar(
                        out=norm[:, s2 * H:(s2 + 1) * H],
                        in0=res[:, s2 * H:(s2 + 1) * H],
                        scalar1=mv[:, s2, 0:1],
                        scalar2=rstd[:, s2:s2 + 1],
                        op0=OP.subtract,
                        op1=OP.mult,
                    )
                nc.vector.tensor_tensor(out=norm, in0=norm, in1=g_sb, op=OP.mult)
                nc.vector.tensor_tensor(out=norm, in0=norm, in1=b_sb, op=OP.add)
                go = opool.tile([128, 2, H], FP32, name="go", tag="go")
                nc.scalar.activation(
                    out=go,
                    in_=norm.rearrange("p (t h) -> p t h", t=2),
                    func=act_func,
                )
                nc.sync.dma_start(
                    out=out[ic * 128:(ic + 1) * 128, 2 * tch: 2 * tch + 2, :],
                    in_=go,
                )

    nc.compile()
    return nc


def _gelu(v):
    from scipy.special import erf
    return 0.5 * v * (1.0 + erf(v / np.sqrt(2.0)))


def _sigmoid(v):
    return 1.0 / (1.0 + np.exp(-v))


def host_prep(x, A_list, in_w, in_b, out_w, out_b, lag_embed,
              ctx_w1, ctx_b1, ctx_w2, ctx_b2,
              gate_w1, gate_b1, gate_w2, gate_b2, ln_g, ln_b):
    f32 = np.float32
    x = np.asarray(x, f32)
    A_list = np.asarray(A_list, f32)
    in_w = np.asarray(in_w, f32)
    in_b = np.asarray(in_b, f32)
    out_w = np.asarray(out_w, f32)
    out_b = np.asarray(out_b, f32)
    lag_embed = np.asarray(lag_embed, f32)
    ctx_w1 = np.asarray(ctx_w1, f32)
    ctx_b1 = np.asarray(ctx_b1, f32)
    ctx_w2 = np.asarray(ctx_w2, f32)
    ctx_b2 = np.asarray(ctx_b2, f32)
    gate_w1 = np.asarray(gate_w1, f32)
    gate_b1 = np.asarray(gate_b1, f32)
    gate_w2 = np.asarray(gate_w2, f32)
    gate_b2 = np.asarray(gate_b2, f32)
    ln_g = np.asarray(ln_g, f32)
    ln_b = np.asarray(ln_b, f32)

    A = A_list / np.maximum(A_list.sum(-1, keepdims=True), np.float32(1e-8))
    ctxm = x.mean(axis=(1, 2))                                   # [B, F]
    cf = _gelu(ctxm @ ctx_w1 + ctx_b1) @ ctx_w2 + ctx_b2         # [B, E]
    lag = lag_embed[:K]                                          # [K, E]
    gi = np.concatenate(
        [np.broadcast_to(lag[None], (B, K, lag.shape[-1])),
         np.broadcast_to(cf[:, None, :], (B, K, cf.shape[-1]))], axis=-1)
    alpha = _sigmoid((_gelu(gi @ gate_w1 + gate_b1) @ gate_w2 + gate_b2)[..., 0])

    W2 = (in_w.astype(np.float64) @ out_w.astype(np.float64)).astype(f32)
    c2 = in_b @ out_w                                            # [H]
    bias_hh = in_b + out_b                                       # [H]

    w2_16 = np.ascontiguousarray(W2).astype(np.float16)
    inw_16 = np.ascontiguousarray(in_w).astype(np.float16)
    gvec = np.ascontiguousarray(
        np.broadcast_to(np.tile(ln_g, 2)[None, :], (128, 2 * H))).astype(f32)
    bvec = np.ascontiguousarray(
        np.broadcast_to(np.tile(ln_b, 2)[None, :], (128, 2 * H))).astype(f32)

    in_maps = []
    for b in range(B):
        ahT = np.ascontiguousarray(
            (alpha[b][:, None, None] * A).transpose(2, 0, 1).reshape(N, K * N)
        ).astype(np.float16)
        for half in range(2):
            T0 = half * TL
            xt = np.zeros((F, TP, N), f32)
            lo = T0 - HALO
            src = max(lo, 0)
            xt[:, src - lo:, :] = x[b, :, src:T0 + TL, :].transpose(2, 1, 0)
            s_t = np.array(
                [alpha[b, :min(T0 + tl, K - 1) + 1].sum() for tl in range(16)], f32)
            cv = s_t[:, None] * c2[None, :] + bias_hh[None, :]   # [16, H]
            cvec = np.ascontiguousarray(
                np.broadcast_to(cv.reshape(1, 16 * H), (128, 16 * H))).astype(f32)
            in_maps.append({
                "xT": np.ascontiguousarray(xt.reshape(F, NTC)).astype(np.float16),
                "ahatT": ahT,
                "w2": w2_16,
                "inw": inw_16,
                "cvec": cvec,
                "gvec": gvec,
                "bvec": bvec,
            })
    return in_maps


def gather(results):
    out = np.empty((B, N, T, H), np.float32)
    for ci in range(NCORES):
        b, half = divmod(ci, 2)
        out[b, :, half * TL:(half + 1) * TL, :] = results[ci]["out"]
    return out


_NC_CACHE = []


def kernel(**inputs) -> np.ndarray:
    in_maps = host_prep(**inputs)
    if not _NC_CACHE:
        _NC_CACHE.append(build_nc())
    nc = _NC_CACHE[0]
    r = run_bass_kernel_spmd(nc, in_maps, list(range(NCORES)))
    return gather(r.results)


# revision 7
# speedup vs baseline: 1.1290x; 1.1290x over previous
"""Trainium2 Bass kernel for nn_PhysicsGuidedGCN.

Reference (x [B=4, N=256, T=128, F=64], A_list [K=8, N, N]):
    A    = row_normalize(A_list)
    h    = x @ in_w + in_b                       # [B,N,T,H]
    alpha= tiny gating MLP over (lag_embed, mean(x))    # [B,K]
    out  = sum_k alpha[:,k] * A[k] @ shift_k(h)  (shift along T, zero fill)
    out  = out @ out_w + out_b
    res  = out + h ; LayerNorm(H) ; gelu         # -> [B,N,T,H]

Key reassociation: with W2 = in_w @ out_w,
    sum_k a_k (A_k x[t-k]) @ W2 = (sum_k a_k A_k x[t-k]) @ W2
so the K lag GEMMs contract F=64-wide x instead of H=256-wide h: 4x fewer
FLOPs.  All bias terms (in_b, out_b, and the A-row-sum identity acting on
in_b@out_w) collapse into a single per-(b,t) vector cvec added before the
LayerNorm; cvec/ln scale/shift stages are skipped entirely when those inputs
are zero/one (true for this model's initialization).

Device pipeline per core (data-parallel over B x T-halves; core = b*2+half,
local T range TL=64 plus HALO=7 backward halo):
  1. agg[i,(t,f)] = sum_k a_k A_k x[j,t-k,f]: 16 matmuls per 8-t block into
     PSUM (lhsT = alpha-scaled A^T slices, rhs = shifted x windows), copy to
     SBUF fp16.
  2. per t: PE-transpose agg[:, t] ([128 i, 64 f] -> [64 f, 128 i]).
  3. per 2-t chunk: fin = aggT.T @ W2 + xT.T @ in_w accumulated in PSUM
     [128, 512], then LayerNorm (bn_stats/bn_aggr) + gelu, DMA out.
The tiny gating MLP (32 scalars) and A normalization/scaling run on host;
they are O(K N^2 + B F) and replicated per the sharding hint.
"""

import numpy as np
from contextlib import ExitStack

import concourse.bass as bass
import concourse.mybir as mybir
import concourse.tile as tile
from concourse import bacc
from concourse.masks import make_identity
from concourse.bass_utils import run_bass_kernel_spmd

B, N, T, F, K, H = 4, 256, 128, 64, 8, 256
TL = 64           # timesteps per core
HALO = K - 1      # backward halo
TP = TL + HALO    # 71 timesteps of x per core
NCORES = 8
FP16 = mybir.dt.float16
FP32 = mybir.dt.float32
AF = mybir.ActivationFunctionType
OP = mybir.AluOpType


def build_nc(with_cvec=False, with_gb=False, act_func=AF.Gelu):
    nc = bacc.Bacc("TRN2", target_bir_lowering=False)
    x_nm = nc.dram_tensor("x_nm", [N, TP * F], FP16, kind="ExternalInput")
    xT = nc.dram_tensor("xT", [F, TP * N], FP16, kind="ExternalInput")
    ahatT = nc.dram_tensor("ahatT", [N, K * N], FP16, kind="ExternalInput")
    w2 = nc.dram_tensor("w2", [F, H], FP16, kind="ExternalInput")
    inw = nc.dram_tensor("inw", [F, H], FP16, kind="ExternalInput")
    if with_cvec:
        cvec = nc.dram_tensor("cvec", [128, 16 * H], FP32, kind="ExternalInput")
    if with_gb:
        gvec = nc.dram_tensor("gvec", [128, 2 * H], FP32, kind="ExternalInput")
        bvec = nc.dram_tensor("bvec", [128, 2 * H], FP32, kind="ExternalInput")
    out = nc.dram_tensor("out", [N, TL, H], FP32, kind="ExternalOutput")

    with tile.TileContext(nc) as tc, ExitStack() as ctx:
        singles = ctx.enter_context(tc.tile_pool(name="singles", bufs=1))
        psA = ctx.enter_context(tc.tile_pool(name="psA", bufs=2, space="PSUM"))
        psT = ctx.enter_context(tc.tile_pool(name="psT", bufs=2, space="PSUM"))
        psF = ctx.enter_context(tc.tile_pool(name="psF", bufs=4, space="PSUM"))
        aggp = ctx.enter_context(tc.tile_pool(name="aggp", bufs=3))
        aggTp = ctx.enter_context(tc.tile_pool(name="aggTp", bufs=6))
        epool = ctx.enter_context(tc.tile_pool(name="epool", bufs=3))
        spool = ctx.enter_context(tc.tile_pool(name="spool", bufs=4))
        opool = ctx.enter_context(tc.tile_pool(name="opool", bufs=3))

        xnm_sb = []
        for jc in range(2):
            t_ = singles.tile([128, TP * F], FP16, name=f"xnm{jc}", tag=f"xnm{jc}")
            nc.sync.dma_start(out=t_, in_=x_nm[jc * 128:(jc + 1) * 128, :])
            xnm_sb.append(t_)
        a_sb = []
        for jc in range(2):
            t_ = singles.tile([128, K * N], FP16, name=f"a_sb{jc}", tag=f"a_sb{jc}")
            nc.sync.dma_start(out=t_, in_=ahatT[jc * 128:(jc + 1) * 128, :])
            a_sb.append(t_)
        xT_sb = singles.tile([F, TP * N], FP16)
        for q in range(4):
            cols = TP * N // 4
            nc.sync.dma_start(out=xT_sb[:, q * cols:(q + 1) * cols],
                              in_=xT[:, q * cols:(q + 1) * cols])
        w2_sb = singles.tile([F, H], FP16)
        nc.sync.dma_start(out=w2_sb, in_=w2[:, :])
        inw_sb = singles.tile([F, H], FP16)
        nc.sync.dma_start(out=inw_sb, in_=inw[:, :])
        if with_cvec:
            c_sb = singles.tile([128, 16 * H], FP32)
            nc.sync.dma_start(out=c_sb, in_=cvec[:, :])
        if with_gb:
            g_sb = singles.tile([128, 2 * H], FP32)
            nc.sync.dma_start(out=g_sb, in_=gvec[:, :])
            b_sb = singles.tile([128, 2 * H], FP32)
            nc.sync.dma_start(out=b_sb, in_=bvec[:, :])
        eps_sb = singles.tile([128, 1], FP32)
        nc.vector.memset(eps_sb, 1e-5)
        ident = singles.tile([128, 128], FP16)
        make_identity(nc, ident)

        for ic in range(2):
            for t8 in range(8):
                # aggregate over lags: agg[i, (8t, 64f)]
                agg_ps = psA.tile([128, 512], FP32, name="agg_ps", tag="agg_ps")
                for k in range(K):
                    for jc in range(2):
                        col = (t8 * 8 + HALO - k) * F
                        nc.tensor.matmul(
                            agg_ps,
                            lhsT=a_sb[jc][:, k * N + ic * 128: k * N + ic * 128 + 128],
                            rhs=xnm_sb[jc][:, col: col + 512],
                            start=(k == 0 and jc == 0),
                            stop=(k == K - 1 and jc == 1),
                        )
                agg_sb = aggp.tile([128, 512], FP16, name="agg_sb", tag="agg_sb")
                nc.vector.tensor_copy(out=agg_sb, in_=agg_ps)

                for tp_ in range(4):
                    aggT = []
                    for trel in range(2):
                        tq = tp_ * 2 + trel
                        aggT_ps = psT.tile([64, 128], FP16, name="aggT_ps",
                                           tag="aggT_ps")
                        nc.tensor.transpose(
                            aggT_ps, agg_sb[:, tq * F:(tq + 1) * F], ident)
                        aggT_sb = aggTp.tile([64, 128], FP16, name="aggT_sb",
                                             tag="aggT_sb")
                        nc.scalar.activation(out=aggT_sb, in_=aggT_ps, func=AF.Copy)
                        aggT.append(aggT_sb)

                    fin = psF.tile([128, 512], FP32, name="fin", tag="fin")
                    for trel in range(2):
                        tloc = t8 * 8 + tp_ * 2 + trel
                        nc.tensor.matmul(
                            fin[:, trel * H:(trel + 1) * H],
                            lhsT=aggT[trel],
                            rhs=w2_sb[:, :],
                            start=True,
                            stop=False,
                            skip_group_check=True,
                        )
                        nc.tensor.matmul(
                            fin[:, trel * H:(trel + 1) * H],
                            lhsT=xT_sb[:, (tloc + HALO) * N + ic * 128:
                                       (tloc + HALO) * N + ic * 128 + 128],
                            rhs=inw_sb[:, :],
                            start=False,
                            stop=True,
                            skip_group_check=True,
                        )

                    # epilogue: (+cvec) LayerNorm (*g+b) gelu -> DMA
                    tch = t8 * 4 + tp_
                    if with_cvec:
                        ccol = 2 * tch * H if tch < 8 else 14 * H
                        res = epool.tile([128, 512], FP32, name="res", tag="res")
                        nc.vector.tensor_tensor(
                            out=res, in0=fin, in1=c_sb[:, ccol: ccol + 512],
                            op=OP.add)
                    else:
                        res = fin
                    stats = spool.tile([128, 2, 6], FP32, name="stats", tag="stats")
                    mv = spool.tile([128, 2, 2], FP32, name="mv", tag="mv")
                    for s2 in range(2):
                        nc.vector.bn_stats(out=stats[:, s2],
                                           in_=res[:, s2 * H:(s2 + 1) * H])
                        nc.vector.bn_aggr(out=mv[:, s2], in_=stats[:, s2])
                    std = spool.tile([128, 2], FP32, name="std", tag="std")
                    nc.scalar.activation(out=std, in_=mv[:, :, 1], func=AF.Sqrt,
                                         bias=eps_sb, scale=1.0)
                    rstd = spool.tile([128, 2], FP32, name="rstd", tag="rstd")
                    nc.vector.reciprocal(out=rstd, in_=std)
                    norm = epool.tile([128, 512], FP32, name="norm", tag="norm")
                    for s2 in range(2):
                        nc.vector.tensor_scalar(
                            out=norm[:, s2 * H:(s2 + 1) * H],
                            in0=res[:, s2 * H:(s2 + 1) * H],
                            scalar1=mv[:, s2, 0:1],
                            scalar2=rstd[:, s2:s2 + 1],
                            op0=OP.subtract,
                            op1=OP.mult,
                        )
                    if with_gb:
                        nc.vector.tensor_tensor(out=norm, in0=norm, in1=g_sb,
                                                op=OP.mult)
                        nc.vector.tensor_tensor(out=norm, in0=norm, in1=b_sb,
                                                op=OP.add)
                    go = opool.tile([128, 2, H], FP32, name="go", tag="go")
                    nc.scalar.activation(
                        out=go, in_=norm.rearrange("p (t h) -> p t h", t=2),
                        func=act_func)
                    nc.sync.dma_start(
                        out=out[ic * 128:(ic + 1) * 128, 2 * tch: 2 * tch + 2, :],
                        in_=go)

    nc.compile()
    return nc


def _gelu(v):
    from scipy.special import erf
    return 0.5 * v * (1.0 + erf(v / np.sqrt(2.0)))


def _sigmoid(v):
    return 1.0 / (1.0 + np.exp(-v))


def host_prep(x, A_list, in_w, in_b, out_w, out_b, lag_embed,
              ctx_w1, ctx_b1, ctx_w2, ctx_b2,
              gate_w1, gate_b1, gate_w2, gate_b2, ln_g, ln_b):
    f32 = np.float32
    x = np.asarray(x, f32)
    A_list = np.asarray(A_list, f32)
    in_w = np.asarray(in_w, f32)
    in_b = np.asarray(in_b, f32)
    out_w = np.asarray(out_w, f32)
    out_b = np.asarray(out_b, f32)
    lag_embed = np.asarray(lag_embed, f32)
    ctx_w1 = np.asarray(ctx_w1, f32)
    ctx_b1 = np.asarray(ctx_b1, f32)
    ctx_w2 = np.asarray(ctx_w2, f32)
    ctx_b2 = np.asarray(ctx_b2, f32)
    gate_w1 = np.asarray(gate_w1, f32)
    gate_b1 = np.asarray(gate_b1, f32)
    gate_w2 = np.asarray(gate_w2, f32)
    gate_b2 = np.asarray(gate_b2, f32)
    ln_g = np.asarray(ln_g, f32)
    ln_b = np.asarray(ln_b, f32)

    A = A_list / np.maximum(A_list.sum(-1, keepdims=True), np.float32(1e-8))
    ctxm = x.mean(axis=(1, 2))                                   # [B, F]
    cf = _gelu(ctxm @ ctx_w1 + ctx_b1) @ ctx_w2 + ctx_b2         # [B, E]
    lag = lag_embed[:K]                                          # [K, E]
    gi = np.concatenate(
        [np.broadcast_to(lag[None], (B, K, lag.shape[-1])),
         np.broadcast_to(cf[:, None, :], (B, K, cf.shape[-1]))], axis=-1)
    alpha = _sigmoid((_gelu(gi @ gate_w1 + gate_b1) @ gate_w2 + gate_b2)[..., 0])

    W2 = (in_w.astype(np.float64) @ out_w.astype(np.float64)).astype(f32)
    c2 = in_b @ out_w                                            # [H]
    bias_hh = in_b + out_b                                       # [H]

    with_gb = not (np.all(ln_g == 1.0) and np.all(ln_b == 0.0))
    w2_16 = np.ascontiguousarray(W2).astype(np.float16)
    inw_16 = np.ascontiguousarray(in_w).astype(np.float16)
    gvec = np.ascontiguousarray(
        np.broadcast_to(np.tile(ln_g, 2)[None, :], (128, 2 * H))).astype(f32)
    bvec = np.ascontiguousarray(
        np.broadcast_to(np.tile(ln_b, 2)[None, :], (128, 2 * H))).astype(f32)

    in_maps = []
    any_cvec = False
    for b in range(B):
        ahT = np.ascontiguousarray(
            (alpha[b][:, None, None] * A).transpose(2, 0, 1).reshape(N, K * N)
        ).astype(np.float16)
        for half in range(2):
            T0 = half * TL
            lo = T0 - HALO
            src = max(lo, 0)
            xn = np.zeros((N, TP, F), f32)
            xn[:, src - lo:, :] = x[b, :, src:T0 + TL, :]
            xt = np.zeros((F, TP, N), f32)
            xt[:, src - lo:, :] = x[b, :, src:T0 + TL, :].transpose(2, 1, 0)
            s_t = np.array(
                [alpha[b, :min(T0 + tl, K - 1) + 1].sum() for tl in range(16)], f32)
            cv = s_t[:, None] * c2[None, :] + bias_hh[None, :]   # [16, H]
            if np.any(cv != 0.0):
                any_cvec = True
            cvec = np.ascontiguousarray(
                np.broadcast_to(cv.reshape(1, 16 * H), (128, 16 * H))).astype(f32)
            in_maps.append({
                "x_nm": np.ascontiguousarray(xn.reshape(N, TP * F)).astype(np.float16),
                "xT": np.ascontiguousarray(xt.reshape(F, TP * N)).astype(np.float16),
                "ahatT": ahT,
                "w2": w2_16,
                "inw": inw_16,
                "cvec": cvec,
                "gvec": gvec,
                "bvec": bvec,
            })
    flags = {"with_cvec": any_cvec, "with_gb": with_gb}
    drop = []
    if not any_cvec:
        drop.append("cvec")
    if not with_gb:
        drop.extend(["gvec", "bvec"])
    for m in in_maps:
        for d in drop:
            del m[d]
    return in_maps, flags


def gather(results):
    out = np.empty((B, N, T, H), np.float32)
    for ci in range(NCORES):
        b, half = divmod(ci, 2)
        out[b, :, half * TL:(half + 1) * TL, :] = results[ci]["out"]
    return out


_NC_CACHE = {}


def get_nc(flags):
    key = (flags["with_cvec"], flags["with_gb"])
    if key not in _NC_CACHE:
        _NC_CACHE[key] = build_nc(with_cvec=key[0], with_gb=key[1])
    return _NC_CACHE[key]


def kernel(**inputs) -> np.ndarray:
    in_maps, flags = host_prep(**inputs)
    nc = get_nc(flags)
    r = run_bass_kernel_spmd(nc, in_maps, list(range(NCORES)))
    return gather(r.results)


# revision 13
# speedup vs baseline: 1.4866x; 1.3167x over previous
"""Trainium2 Bass kernel for nn_PhysicsGuidedGCN.

Reference (x [B=4, N=256, T=128, F=64], A_list [K=8, N, N]):
    A    = row_normalize(A_list)
    h    = x @ in_w + in_b                       # [B,N,T,H]
    alpha= tiny gating MLP over (lag_embed, mean(x))    # [B,K]
    out  = sum_k alpha[:,k] * A[k] @ shift_k(h)  (shift along T, zero fill)
    out  = out @ out_w + out_b
    res  = out + h ; LayerNorm(H) ; gelu         # -> [B,N,T,H]

Key reassociation: with W2 = in_w @ out_w,
    sum_k a_k (A_k x[t-k]) @ W2 = (sum_k a_k A_k x[t-k]) @ W2
so the K lag GEMMs contract F=64-wide x instead of H=256-wide h: 4x fewer
FLOPs.  All bias terms (in_b, out_b, and the A-row-sum identity acting on
in_b@out_w) collapse into a single per-(b,t) vector cvec added before the
LayerNorm; cvec/ln scale/shift stages are skipped entirely when those inputs
are zero/one (true for this model's initialization).

Device pipeline per core (data-parallel over B x T-halves; core = b*2+half,
local T range TL=64 plus HALO=7 backward halo):
  1. agg[i,(t,f)] = sum_k a_k A_k x[j,t-k,f]: 16 matmuls per 8-t block into
     PSUM (lhsT = alpha-scaled A^T slices, rhs = shifted x windows), copy to
     SBUF fp16.
  2. per t: PE-transpose agg[:, t] ([128 i, 64 f] -> [64 f, 128 i]).
  3. per 2-t chunk: fin = aggT.T @ W2 + xT.T @ in_w accumulated in PSUM
     [128, 512], then LayerNorm (bn_stats/bn_aggr) + gelu, DMA out.
The tiny gating MLP (32 scalars) and A normalization/scaling run on host;
they are O(K N^2 + B F) and replicated per the sharding hint.
"""

import numpy as np
from contextlib import ExitStack

import concourse.bass as bass
import concourse.mybir as mybir
import concourse.tile as tile
from concourse import bacc
from concourse.masks import make_identity
from concourse.bass_utils import run_bass_kernel_spmd

B, N, T, F, K, H = 4, 256, 128, 64, 8, 256
TL = 64           # timesteps per core
HALO = K - 1      # backward halo
TP = TL + HALO    # 71 timesteps of x per core
NCORES = 8
GROUP = 16        # output chunks per batched-sqrt group (amortizes ACT
                  # activation-table reloads between Sqrt and Gelu)
FP16 = mybir.dt.float16
FP32 = mybir.dt.float32
AF = mybir.ActivationFunctionType
OP = mybir.AluOpType


def build_nc(with_cvec=False, with_gb=False, act_func=AF.Gelu):
    nc = bacc.Bacc("TRN2", target_bir_lowering=False)
    x_nm = nc.dram_tensor("x_nm", [N, TP * F], FP16, kind="ExternalInput")
    xT = nc.dram_tensor("xT", [F, TP * N], FP16, kind="ExternalInput")
    ahatT = nc.dram_tensor("ahatT", [N, K * N], FP16, kind="ExternalInput")
    w2 = nc.dram_tensor("w2", [F, H], FP16, kind="ExternalInput")
    inw = nc.dram_tensor("inw", [F, H], FP16, kind="ExternalInput")
    if with_cvec:
        cvec = nc.dram_tensor("cvec", [128, 16 * H], FP32, kind="ExternalInput")
    if with_gb:
        gvec = nc.dram_tensor("gvec", [128, 2 * H], FP32, kind="ExternalInput")
        bvec = nc.dram_tensor("bvec", [128, 2 * H], FP32, kind="ExternalInput")
    out = nc.dram_tensor("out", [N, TL, H], FP32, kind="ExternalOutput")

    with tile.TileContext(nc) as tc, ExitStack() as ctx:
        singles = ctx.enter_context(tc.tile_pool(name="singles", bufs=1))
        psA = ctx.enter_context(tc.tile_pool(name="psA", bufs=2, space="PSUM"))
        psT = ctx.enter_context(tc.tile_pool(name="psT", bufs=2, space="PSUM"))
        psF = ctx.enter_context(tc.tile_pool(name="psF", bufs=4, space="PSUM"))
        aggp = ctx.enter_context(tc.tile_pool(name="aggp", bufs=3))
        aggTp = ctx.enter_context(tc.tile_pool(name="aggTp", bufs=6))
        epool = ctx.enter_context(tc.tile_pool(name="epool", bufs=3))
        npool = ctx.enter_context(tc.tile_pool(name="npool", bufs=GROUP + 2))
        spool = ctx.enter_context(tc.tile_pool(name="spool", bufs=4))
        gpool = ctx.enter_context(tc.tile_pool(name="gpool", bufs=2))
        opool = ctx.enter_context(tc.tile_pool(name="opool", bufs=4))

        xnm_sb = []
        for jc in range(2):
            t_ = singles.tile([128, TP * F], FP16, name=f"xnm{jc}", tag=f"xnm{jc}")
            nc.sync.dma_start(out=t_, in_=x_nm[jc * 128:(jc + 1) * 128, :])
            xnm_sb.append(t_)
        a_sb = []
        for jc in range(2):
            t_ = singles.tile([128, K * N], FP16, name=f"a_sb{jc}", tag=f"a_sb{jc}")
            nc.sync.dma_start(out=t_, in_=ahatT[jc * 128:(jc + 1) * 128, :])
            a_sb.append(t_)
        xT_sb = singles.tile([F, TP * N], FP16)
        for q in range(4):
            cols = TP * N // 4
            nc.sync.dma_start(out=xT_sb[:, q * cols:(q + 1) * cols],
                              in_=xT[:, q * cols:(q + 1) * cols])
        w2_sb = singles.tile([F, H], FP16)
        nc.sync.dma_start(out=w2_sb, in_=w2[:, :])
        inw_sb = singles.tile([F, H], FP16)
        nc.sync.dma_start(out=inw_sb, in_=inw[:, :])
        if with_cvec:
            c_sb = singles.tile([128, 16 * H], FP32)
            nc.sync.dma_start(out=c_sb, in_=cvec[:, :])
        if with_gb:
            g_sb = singles.tile([128, 2 * H], FP32)
            nc.sync.dma_start(out=g_sb, in_=gvec[:, :])
            b_sb = singles.tile([128, 2 * H], FP32)
            nc.sync.dma_start(out=b_sb, in_=bvec[:, :])
        eps_sb = singles.tile([128, 1], FP32)
        nc.vector.memset(eps_sb, 1e-5)
        ident = singles.tile([128, 128], FP16)
        make_identity(nc, ident)

        mvg = None
        pending = []

        def flush_group():
            """Batched sqrt over the group's variances, then per-chunk gelu
            (with rstd folded into the activation scale) and output DMA."""
            nonlocal mvg, pending
            stdg = gpool.tile([128, 2 * GROUP, 1], FP32, name="stdg", tag="stdg")
            nc.scalar.activation(out=stdg, in_=mvg[:, :, 1:2], func=AF.Sqrt,
                                 bias=eps_sb, scale=1.0)
            rstdg = gpool.tile([128, 2 * GROUP, 1], FP32, name="rstdg",
                               tag="rstdg")
            nc.vector.reciprocal(out=rstdg, in_=stdg)
            for norm0, slot, oslice in pending:
                go = opool.tile([128, 2, H], FP32, name="go", tag="go")
                if with_gb:
                    # ((res-mu)*rstd)*g + b, then gelu
                    nrm = epool.tile([128, 512], FP32, name="nrm", tag="nrm")
                    for s2 in range(2):
                        nc.vector.tensor_scalar(
                            out=nrm[:, s2 * H:(s2 + 1) * H],
                            in0=norm0[:, s2 * H:(s2 + 1) * H],
                            scalar1=rstdg[:, 2 * slot + s2, 0:1],
                            scalar2=None,
                            op0=OP.mult,
                        )
                    nc.vector.tensor_tensor(out=nrm, in0=nrm, in1=g_sb,
                                            op=OP.mult)
                    nc.vector.tensor_tensor(out=nrm, in0=nrm, in1=b_sb,
                                            op=OP.add)
                    nc.scalar.activation(
                        out=go, in_=nrm.rearrange("p (t h) -> p t h", t=2),
                        func=act_func)
                else:
                    for s2 in range(2):
                        nc.scalar.activation(
                            out=go[:, s2],
                            in_=norm0[:, s2 * H:(s2 + 1) * H],
                            func=act_func,
                            scale=rstdg[:, 2 * slot + s2, 0:1],
                        )
                nc.sync.dma_start(out=oslice, in_=go)
            mvg = None
            pending = []

        for ic in range(2):
            for t8 in range(8):
                # aggregate over lags: agg[i, (8t, 64f)]
                agg_ps = psA.tile([128, 512], FP32, name="agg_ps", tag="agg_ps")
                for k in range(K):
                    for jc in range(2):
                        col = (t8 * 8 + HALO - k) * F
                        nc.tensor.matmul(
                            agg_ps,
                            lhsT=a_sb[jc][:, k * N + ic * 128: k * N + ic * 128 + 128],
                            rhs=xnm_sb[jc][:, col: col + 512],
                            start=(k == 0 and jc == 0),
                            stop=(k == K - 1 and jc == 1),
                        )
                agg_sb = aggp.tile([128, 512], FP16, name="agg_sb", tag="agg_sb")
                nc.vector.tensor_copy(out=agg_sb, in_=agg_ps)

                for tp_ in range(4):
                    aggT = []
                    for trel in range(2):
                        tq = tp_ * 2 + trel
                        aggT_ps = psT.tile([64, 128], FP16, name="aggT_ps",
                                           tag="aggT_ps")
                        nc.tensor.transpose(
                            aggT_ps, agg_sb[:, tq * F:(tq + 1) * F], ident)
                        aggT_sb = aggTp.tile([64, 128], FP16, name="aggT_sb",
                                             tag="aggT_sb")
                        nc.scalar.activation(out=aggT_sb, in_=aggT_ps, func=AF.Copy)
                        aggT.append(aggT_sb)

                    fin = psF.tile([128, 512], FP32, name="fin", tag="fin")
                    for trel in range(2):
                        tloc = t8 * 8 + tp_ * 2 + trel
                        nc.tensor.matmul(
                            fin[:, trel * H:(trel + 1) * H],
                            lhsT=aggT[trel],
                            rhs=w2_sb[:, :],
                            start=True,
                            stop=False,
                            skip_group_check=True,
                        )
                        nc.tensor.matmul(
                            fin[:, trel * H:(trel + 1) * H],
                            lhsT=xT_sb[:, (tloc + HALO) * N + ic * 128:
                                       (tloc + HALO) * N + ic * 128 + 128],
                            rhs=inw_sb[:, :],
                            start=False,
                            stop=True,
                            skip_group_check=True,
                        )

                    # epilogue: (+cvec) stats + mean-subtract now (frees PSUM);
                    # sqrt batched per GROUP, rstd folded into the gelu scale.
                    tch = t8 * 4 + tp_
                    ci = ic * 32 + tch
                    slot = ci % GROUP
                    if with_cvec:
                        ccol = 2 * tch * H if tch < 8 else 14 * H
                        res = epool.tile([128, 512], FP32, name="res", tag="res")
                        nc.vector.tensor_tensor(
                            out=res, in0=fin, in1=c_sb[:, ccol: ccol + 512],
                            op=OP.add)
                    else:
                        res = fin
                    if mvg is None:
                        mvg = gpool.tile([128, 2 * GROUP, 2], FP32, name="mvg",
                                         tag="mvg")
                    stats = spool.tile([128, 2, 6], FP32, name="stats", tag="stats")
                    norm0 = npool.tile([128, 512], FP32, name="norm0", tag="norm0")
                    for s2 in range(2):
                        nc.vector.bn_stats(out=stats[:, s2],
                                           in_=res[:, s2 * H:(s2 + 1) * H])
                        nc.vector.bn_aggr(out=mvg[:, 2 * slot + s2],
                                          in_=stats[:, s2])
                        nc.vector.tensor_scalar(
                            out=norm0[:, s2 * H:(s2 + 1) * H],
                            in0=res[:, s2 * H:(s2 + 1) * H],
                            scalar1=mvg[:, 2 * slot + s2, 0:1],
                            scalar2=None,
                            op0=OP.subtract,
                        )
                    pending.append(
                        (norm0, slot,
                         out[ic * 128:(ic + 1) * 128, 2 * tch: 2 * tch + 2, :]))
                    if slot == GROUP - 1:
                        flush_group()

    nc.compile()
    return nc


def _gelu(v):
    from scipy.special import erf
    return 0.5 * v * (1.0 + erf(v / np.sqrt(2.0)))


def _sigmoid(v):
    return 1.0 / (1.0 + np.exp(-v))


def host_prep(x, A_list, in_w, in_b, out_w, out_b, lag_embed,
              ctx_w1, ctx_b1, ctx_w2, ctx_b2,
              gate_w1, gate_b1, gate_w2, gate_b2, ln_g, ln_b):
    f32 = np.float32
    x = np.asarray(x, f32)
    A_list = np.asarray(A_list, f32)
    in_w = np.asarray(in_w, f32)
    in_b = np.asarray(in_b, f32)
    out_w = np.asarray(out_w, f32)
    out_b = np.asarray(out_b, f32)
    lag_embed = np.asarray(lag_embed, f32)
    ctx_w1 = np.asarray(ctx_w1, f32)
    ctx_b1 = np.asarray(ctx_b1, f32)
    ctx_w2 = np.asarray(ctx_w2, f32)
    ctx_b2 = np.asarray(ctx_b2, f32)
    gate_w1 = np.asarray(gate_w1, f32)
    gate_b1 = np.asarray(gate_b1, f32)
    gate_w2 = np.asarray(gate_w2, f32)
    gate_b2 = np.asarray(gate_b2, f32)
    ln_g = np.asarray(ln_g, f32)
    ln_b = np.asarray(ln_b, f32)

    A = A_list / np.maximum(A_list.sum(-1, keepdims=True), np.float32(1e-8))
    ctxm = x.mean(axis=(1, 2))                                   # [B, F]
    cf = _gelu(ctxm @ ctx_w1 + ctx_b1) @ ctx_w2 + ctx_b2         # [B, E]
    lag = lag_embed[:K]                                          # [K, E]
    gi = np.concatenate(
        [np.broadcast_to(lag[None], (B, K, lag.shape[-1])),
         np.broadcast_to(cf[:, None, :], (B, K, cf.shape[-1]))], axis=-1)
    alpha = _sigmoid((_gelu(gi @ gate_w1 + gate_b1) @ gate_w2 + gate_b2)[..., 0])

    W2 = (in_w.astype(np.float64) @ out_w.astype(np.float64)).astype(f32)
    c2 = in_b @ out_w                                            # [H]
    bias_hh = in_b + out_b                                       # [H]

    with_gb = not (np.all(ln_g == 1.0) and np.all(ln_b == 0.0))
    w2_16 = np.ascontiguousarray(W2).astype(np.float16)
    inw_16 = np.ascontiguousarray(in_w).astype(np.float16)
    gvec = np.ascontiguousarray(
        np.broadcast_to(np.tile(ln_g, 2)[None, :], (128, 2 * H))).astype(f32)
    bvec = np.ascontiguousarray(
        np.broadcast_to(np.tile(ln_b, 2)[None, :], (128, 2 * H))).astype(f32)

    in_maps = []
    any_cvec = False
    for b in range(B):
        ahT = np.ascontiguousarray(
            (alpha[b][:, None, None] * A).transpose(2, 0, 1).reshape(N, K * N)
        ).astype(np.float16)
        for half in range(2):
            T0 = half * TL
            lo = T0 - HALO
            src = max(lo, 0)
            xn = np.zeros((N, TP, F), f32)
            xn[:, src - lo:, :] = x[b, :, src:T0 + TL, :]
            xt = np.zeros((F, TP, N), f32)
            xt[:, src - lo:, :] = x[b, :, src:T0 + TL, :].transpose(2, 1, 0)
            s_t = np.array(
                [alpha[b, :min(T0 + tl, K - 1) + 1].sum() for tl in range(16)], f32)
            cv = s_t[:, None] * c2[None, :] + bias_hh[None, :]   # [16, H]
            if np.any(cv != 0.0):
                any_cvec = True
            cvec = np.ascontiguousarray(
                np.broadcast_to(cv.reshape(1, 16 * H), (128, 16 * H))).astype(f32)
            in_maps.append({
                "x_nm": np.ascontiguousarray(xn.reshape(N, TP * F)).astype(np.float16),
                "xT": np.ascontiguousarray(xt.reshape(F, TP * N)).astype(np.float16),
                "ahatT": ahT,
                "w2": w2_16,
                "inw": inw_16,
                "cvec": cvec,
                "gvec": gvec,
                "bvec": bvec,
            })
    flags = {"with_cvec": any_cvec, "with_gb": with_gb}
    drop = []
    if not any_cvec:
        drop.append("cvec")
    if not with_gb:
        drop.extend(["gvec", "bvec"])
    for m in in_maps:
        for d in drop:
            del m[d]
    return in_maps, flags


def gather(results):
    out = np.empty((B, N, T, H), np.float32)
    for ci in range(NCORES):
        b, half = divmod(ci, 2)
        out[b, :, half * TL:(half + 1) * TL, :] = results[ci]["out"]
    return out


_NC_CACHE = {}


def get_nc(flags):
    key = (flags["with_cvec"], flags["with_gb"])
    if key not in _NC_CACHE:
        _NC_CACHE[key] = build_nc(with_cvec=key[0], with_gb=key[1])
    return _NC_CACHE[key]


def kernel(**inputs) -> np.ndarray:
    in_maps, flags = host_prep(**inputs)
    nc = get_nc(flags)
    r = run_bass_kernel_spmd(nc, in_maps, list(range(NCORES)))
    return gather(r.results)
